# revision 7
# baseline (speedup 1.0000x reference)
"""Trainium2 Bass kernel for a windowed cross-attention layer — v3.

Same math as the baseline kernel (see kernel.py docstring), restructured
for host<->device transfer, which dominates the per-execution cost in
this environment (~25 GB/s in, ~16 GB/s out, plus per-tensor overhead):

  - 2 input tensors per core instead of 11+:
      xn  bf16 [1280, 1024]: rows 0:1024 hidden (natural layout),
          rows 1024:1152 identity (for PE transpose), rows 1152:1280 masks.
      f8  fp8e3m4 [6144, 1024]: cross^T (true scale), then
          Wq^T/Wk^T/Wv^T/Wg^T/Wo^T each pre-scaled by 64 (fp8e3 normal
          range); upcast on-chip to bf16 with a 1/64 factor folded in.
  - hidden^T for the Q/G projections is derived on-chip from xn via
    DMA-transpose (XBAR), so the hidden tensor is sent only once.
  - output is written bf16 and upcast host-side.
  - out-projection runs in bf16 (was fp32): 4x fewer PE cycles.

Sharding: data-parallel over batch. B == 8 == n_cores, one batch element
per NeuronCore, weights replicated, no collectives.
"""

import numpy as np

import concourse.bacc as bacc
import concourse.mybir as mybir
from concourse import tile
from concourse.bass_utils import run_bass_kernel_spmd

B, S, H, NH = 8, 1024, 1024, 16
HD = H // NH            # 64
WIN = 128
HW_ = WIN // 2          # 64  (window half-width)
SCALE = float(HD) ** -0.5
NCORES = 8
PT = 128                # partition tile
NT = H // PT            # 8
KPAD = S + 2 * HW_      # 1152 (left/right zero pads for the key band)
JB = 2 * WIN            # 256: key-band width per 128-query tile
LN_EPS = 1e-5
WSCALE = 64.0           # host-side fp8 scale-row pre-scale
WINV = float(1.0 / WSCALE)
NMAT = 5                # q, k, v, g, o

F32 = mybir.dt.float32
BF16 = mybir.dt.bfloat16
FP8 = mybir.dt.float8e3
U8 = mybir.dt.uint8
NPBF16 = mybir.dt.np(BF16)
NPFP8 = mybir.dt.np(FP8)

AF = mybir.ActivationFunctionType
ALU = mybir.AluOpType
AX = mybir.AxisListType

_PROGRAM_CACHE: dict = {}

# w4 pack row bases (int4 weights, W.T nibble-packed along out pairs)
M_Q, M_K, M_V, M_G, M_O = range(NMAT)
R_SC = H  # f8 scale rows base (rows H..H+NMAT)


def _build_program(use_bq: bool, use_bg: bool, use_bo: bool):
    nc = bacc.Bacc("TRN2", target_bir_lowering=False, debug=False)

    xn = nc.dram_tensor("xn", [S, H], BF16, kind="ExternalInput")
    f8 = nc.dram_tensor("f8", [H + NMAT, H], FP8, kind="ExternalInput")
    w4 = nc.dram_tensor("w4", [NMAT * H, H // 2], U8, kind="ExternalInput")
    use_smalls = use_bq or use_bg or use_bo
    if use_smalls:
        # [:, 0:8] SCALE*bq per out-tile, [:, 8:1032] bg bcast, [:, 1032:2056] bo_eff bcast
        smalls = nc.dram_tensor("smalls", [PT, 2056], F32, kind="ExternalInput")
    outp = nc.dram_tensor("out", [S, H], BF16, kind="ExternalOutput")

    with tile.TileContext(nc) as tc:
        with (
            tc.tile_pool(name="consts", bufs=1) as cpool,
            tc.tile_pool(name="ctxp", bufs=1) as ctxpool,
            tc.tile_pool(name="t1p", bufs=1) as t1pool,
        ):
            # masks + identity are generated on-chip (affine band predicates)
            mask_sb = cpool.tile([PT, 3 * JB], BF16, tag="mask")
            mid = mask_sb[:, JB:2 * JB]
            nc.gpsimd.memset(mid, 1.0)
            # mid: valid iff 0 <= jj - i <= 128
            nc.gpsimd.affine_select(mid, mid, pattern=[[1, JB]], base=0,
                                    channel_multiplier=-1,
                                    compare_op=ALU.is_ge, fill=0.0)
            nc.gpsimd.affine_select(mid, mid, pattern=[[-1, JB]], base=WIN,
                                    channel_multiplier=1,
                                    compare_op=ALU.is_ge, fill=0.0)
            # left tile: also jj >= 64 ; right tile: also jj <= 191
            nc.gpsimd.affine_select(mask_sb[:, 0:JB], mid, pattern=[[1, JB]],
                                    base=-HW_, channel_multiplier=0,
                                    compare_op=ALU.is_ge, fill=0.0)
            nc.gpsimd.affine_select(mask_sb[:, 2 * JB:3 * JB], mid,
                                    pattern=[[-1, JB]], base=(JB - HW_ - 1),
                                    channel_multiplier=0,
                                    compare_op=ALU.is_ge, fill=0.0)
            iden_sb = cpool.tile([PT, PT], BF16, tag="iden")
            nc.gpsimd.memset(iden_sb[:], 1.0)
            nc.gpsimd.affine_select(iden_sb[:], iden_sb[:], pattern=[[1, PT]],
                                    base=0, channel_multiplier=-1,
                                    compare_op=ALU.is_ge, fill=0.0)
            nc.gpsimd.affine_select(iden_sb[:], iden_sb[:], pattern=[[-1, PT]],
                                    base=0, channel_multiplier=1,
                                    compare_op=ALU.is_ge, fill=0.0)
            # int4 dequant scales: s_sb[:, m*8+i] = scale for (matrix m, tile i)
            s8_sb = cpool.tile([PT, NMAT * NT], FP8, tag="s8")
            nc.sync.dma_start(
                s8_sb[:].rearrange("p (m i) -> p m i", m=NMAT),
                f8.ap()[R_SC:R_SC + NMAT, :].rearrange("m (i p) -> p m i", p=PT))
            s_sb = cpool.tile([PT, NMAT * NT], F32, tag="ssc")
            nc.gpsimd.tensor_scalar_mul(s_sb[:], s8_sb[:], WINV)
            m8_sb = cpool.tile([PT, NMAT * NT], F32, tag="m8sc")
            nc.gpsimd.tensor_scalar_mul(m8_sb[:], s_sb[:], -8.0)

            def unpack_w4(m, i, dst_tile, stpool):
                """w4[m] tile i (uint8 nibble pairs) -> dst bf16 [128, H].

                byte b = (n_hi << 4) | n_lo packs out-columns (2f, 2f+1);
                w = (n - 8) * s  with s per in-feature row (partition).
                """
                sA = s_sb[:, m * NT + i:m * NT + i + 1]
                mA = m8_sb[:, m * NT + i:m * NT + i + 1]
                u8t = stpool.tile([PT, H // 2], U8, tag="u8")
                nc.scalar.dma_start(
                    u8t[:], w4.ap()[m * H + i * PT:m * H + (i + 1) * PT, :])
                d2 = dst_tile[:].rearrange("p (f t) -> p t f", t=2)
                nib = stpool.tile([PT, H // 2], U8, tag="nib")
                nc.gpsimd.tensor_scalar(
                    nib[:], u8t[:], 4, None, op0=ALU.logical_shift_right)
                nc.gpsimd.tensor_scalar(
                    d2[:, 0:1, :], nib[:].rearrange("p (o f) -> p o f", o=1),
                    sA, mA, op0=ALU.mult, op1=ALU.add)
                nib2 = stpool.tile([PT, H // 2], U8, tag="nib")
                nc.gpsimd.tensor_scalar(
                    nib2[:], u8t[:], 15, None, op0=ALU.bitwise_and)
                nc.gpsimd.tensor_scalar(
                    d2[:, 1:2, :], nib2[:].rearrange("p (o f) -> p o f", o=1),
                    sA, mA, op0=ALU.mult, op1=ALU.add)
            if use_smalls:
                sm_sb = cpool.tile([PT, 2056], F32, tag="smalls")
                nc.sync.dma_start(sm_sb[:], smalls.ap()[:])

            ctx_sb = [ctxpool.tile([PT, S], BF16, tag=f"ctx{i}", name=f"ctx{i}")
                      for i in range(NT)]
            t1_sb = [t1pool.tile([PT, H], BF16, tag=f"t1_{i}", name=f"t1_{i}")
                     for i in range(NT)]

            with tc.tile_pool(name="kvpool", bufs=1) as kvpool:
                # K^T padded key band [feature, 64 | tokens | 64]
                kt_sb = [kvpool.tile([PT, KPAD], BF16, tag=f"kt{i}", name=f"kt{i}")
                         for i in range(NT)]
                # V in shifted tiling: vs[u] rows = tokens [128u-64, 128u+64)
                vs_sb = [kvpool.tile([PT, H], BF16, tag=f"vs{i}", name=f"vs{i}")
                         for i in range(NT + 1)]
                for i in range(NT):
                    nc.gpsimd.memset(kt_sb[i][:, 0:HW_], 0.0)
                    nc.gpsimd.memset(kt_sb[i][:, KPAD - HW_:KPAD], 0.0)
                nc.gpsimd.memset(vs_sb[0][0:HW_, :], 0.0)
                nc.gpsimd.memset(vs_sb[NT][PT - HW_:PT, :], 0.0)

                # ---- Phase 1: K = cross @ Wk.T (transposed), V (shifted) ----
                with (
                    tc.tile_pool(name="stage8", bufs=1) as spool8,
                    tc.tile_pool(name="ctpool", bufs=1) as ctpool,
                    tc.tile_pool(name="w1", bufs=1) as wpool1,
                    tc.tile_pool(name="ps1", bufs=4, space="PSUM") as ps1,
                ):
                    ct8 = [spool8.tile([PT, H], FP8, tag=f"ct8_{i}", name=f"ct8_{i}")
                           for i in range(NT)]
                    for i in range(NT):
                        nc.sync.dma_start(ct8[i][:], f8.ap()[i * PT:(i + 1) * PT, :])

                    ct_sb = [ctpool.tile([PT, S], BF16, tag=f"ct{i}", name=f"ct{i}")
                             for i in range(NT)]
                    wk_sb = [wpool1.tile([PT, H], BF16, tag=f"wk{i}", name=f"wk{i}")
                             for i in range(NT)]
                    wv_sb = [wpool1.tile([PT, H], BF16, tag=f"wv{i}", name=f"wv{i}")
                             for i in range(NT)]
                    for i in range(NT):
                        nc.gpsimd.tensor_copy(ct_sb[i][:], ct8[i][:])
                        unpack_w4(M_K, i, wk_sb[i], spool8)
                        unpack_w4(M_V, i, wv_sb[i], spool8)

                    # K^T[o, s] = sum_h Wk.T[h, o].T @ cross^T[h, s]
                    for ot in range(NT):
                        for sh in range(2):
                            acc = ps1.tile([PT, 512], F32, tag="ps1")
                            for ht in range(NT):
                                nc.tensor.matmul(
                                    acc[:],
                                    wk_sb[ht][:, ot * PT:(ot + 1) * PT],
                                    ct_sb[ht][:, sh * 512:(sh + 1) * 512],
                                    start=(ht == 0), stop=(ht == NT - 1),
                                )
                            nc.scalar.copy(
                                kt_sb[ot][:, HW_ + sh * 512: HW_ + (sh + 1) * 512],
                                acc[:],
                            )

                    # V[s, o] = cross @ Wv.T, then build the token-shifted
                    # tiles via SBUF->SBUF DMA (compute engines cannot move
                    # data across partition lanes).
                    v_sb = [ctpool.tile([PT, H], BF16, tag=f"v{i}", name=f"v{i}")
                            for i in range(NT)]
                    for st in range(NT):
                        for oh in range(2):
                            acc = ps1.tile([PT, 512], F32, tag="ps1")
                            for ht in range(NT):
                                nc.tensor.matmul(
                                    acc[:],
                                    ct_sb[ht][:, st * PT:(st + 1) * PT],
                                    wv_sb[ht][:, oh * 512:(oh + 1) * 512],
                                    start=(ht == 0), stop=(ht == NT - 1),
                                )
                            nc.scalar.copy(
                                v_sb[st][:, oh * 512:(oh + 1) * 512], acc[:])
                    for u in range(NT + 1):
                        if u > 0:
                            nc.sync.dma_start(
                                vs_sb[u][0:HW_, :], v_sb[u - 1][HW_:PT, :])
                        if u < NT:
                            nc.sync.dma_start(
                                vs_sb[u][HW_:PT, :], v_sb[u][0:HW_, :])

                with tc.tile_pool(name="qpool", bufs=1) as qpool:
                    qt_sb = [qpool.tile([PT, S], BF16, tag=f"qt{i}", name=f"qt{i}")
                             for i in range(NT)]

                    # ---- Phase 2: Q^T (scaled, biased) and gate tanh ----
                    with (
                        tc.tile_pool(name="stage8b", bufs=1) as spool8b,
                        tc.tile_pool(name="xtpool", bufs=1) as xtpool,
                        tc.tile_pool(name="w2", bufs=1) as wpool2,
                        tc.tile_pool(name="ps2", bufs=4, space="PSUM") as ps2,
                        tc.tile_pool(name="gtmp", bufs=3) as gtmp,
                    ):
                        # hidden^T via DMA-transpose from the natural-layout rows
                        xt_sb = [xtpool.tile([PT, S], BF16, tag=f"xt{i}", name=f"xt{i}")
                                 for i in range(NT)]
                        for i in range(NT):
                            nc.sync.dma_start(
                                xt_sb[i][:], xn.ap()[0:S, i * PT:(i + 1) * PT],
                                transpose=True)
                        wq_sb = [wpool2.tile([PT, H], BF16, tag=f"wq{i}", name=f"wq{i}")
                                 for i in range(NT)]
                        wg_sb = [wpool2.tile([PT, H], BF16, tag=f"wg{i}", name=f"wg{i}")
                                 for i in range(NT)]
                        for i in range(NT):
                            unpack_w4(M_Q, i, wq_sb[i], spool8b)
                            unpack_w4(M_G, i, wg_sb[i], spool8b)

                        for ot in range(NT):
                            for sh in range(2):
                                acc = ps2.tile([PT, 512], F32, tag="ps2")
                                for ht in range(NT):
                                    nc.tensor.matmul(
                                        acc[:],
                                        wq_sb[ht][:, ot * PT:(ot + 1) * PT],
                                        xt_sb[ht][:, sh * 512:(sh + 1) * 512],
                                        start=(ht == 0), stop=(ht == NT - 1),
                                    )
                                # q_scaled = SCALE*q (+ SCALE*bq)
                                nc.scalar.activation(
                                    qt_sb[ot][:, sh * 512:(sh + 1) * 512],
                                    acc[:], AF.Identity,
                                    bias=(sm_sb[:, ot:ot + 1] if use_bq else 0.0),
                                    scale=SCALE,
                                )

                        # z[s, o] = hidden @ Wg.T ; t1 = sigmoid(z) via tanh
                        for st in range(NT):
                            for oh in range(2):
                                acc = ps2.tile([PT, 512], F32, tag="ps2")
                                for ht in range(NT):
                                    nc.tensor.matmul(
                                        acc[:],
                                        xt_sb[ht][:, st * PT:(st + 1) * PT],
                                        wg_sb[ht][:, oh * 512:(oh + 1) * 512],
                                        start=(ht == 0), stop=(ht == NT - 1),
                                    )
                                sl = slice(oh * 512, (oh + 1) * 512)
                                if use_bg:
                                    zb = gtmp.tile([PT, 512], F32, tag="zb")
                                    nc.vector.tensor_tensor(
                                        zb[:], acc[:], sm_sb[:, 8 + oh * 512:8 + (oh + 1) * 512],
                                        op=ALU.add)
                                    zin = zb
                                else:
                                    zin = acc
                                th = gtmp.tile([PT, 512], BF16, tag="th")
                                nc.scalar.activation(th[:], zin[:], AF.Tanh, scale=0.5)
                                # gate = sigmoid(z) = 0.5*tanh(z/2) + 0.5
                                nc.vector.tensor_scalar(
                                    t1_sb[st][:, sl], th[:], 0.5, 0.5,
                                    op0=ALU.mult, op1=ALU.add)

                    # ---- Phase 3: windowed attention ----
                    with (
                        tc.tile_pool(name="attn_sb", bufs=3) as apool,
                        tc.tile_pool(name="stats", bufs=4) as spool,
                        tc.tile_pool(name="ps_sc", bufs=2, space="PSUM") as ps_sc,
                        tc.tile_pool(name="ps_at", bufs=2, space="PSUM") as ps_at,
                        tc.tile_pool(name="ps_cx", bufs=2, space="PSUM") as ps_cx,
                    ):
                        for p in range(NT):
                            for t in range(NT):   # query tile
                                mv = 0 if t == 0 else (2 if t == NT - 1 else 1)
                                # separate PSUM tiles per head: the two MMs
                                # use disjoint PE row-groups (partition base
                                # 0 vs 64) and can run concurrently in the
                                # array — concurrent writes to one PSUM bank
                                # are fatal on HW.
                                scs = [ps_sc.tile([PT, JB], F32, tag=f"sc{h}",
                                                  name=f"sc{h}")
                                       for h in range(2)]
                                for hh in range(2):
                                    nc.tensor.matmul(
                                        scs[hh][:],
                                        qt_sb[p][hh * HD:(hh + 1) * HD,
                                                 t * PT:(t + 1) * PT],
                                        kt_sb[p][hh * HD:(hh + 1) * HD,
                                                 t * PT:t * PT + JB],
                                        start=True, stop=True,
                                    )
                                ex = apool.tile([PT, 512], BF16, tag="ex")
                                for hh in range(2):
                                    nc.scalar.activation(
                                        ex[:, hh * JB:(hh + 1) * JB],
                                        scs[hh][:], AF.Exp)
                                am = apool.tile([PT, 512], BF16, tag="am")
                                ssum = spool.tile([PT, 2], F32, tag="ssum")
                                for hh in range(2):
                                    sl = slice(hh * JB, (hh + 1) * JB)
                                    nc.vector.tensor_tensor(
                                        am[:, sl], ex[:, sl],
                                        mask_sb[:, mv * JB:(mv + 1) * JB],
                                        op=ALU.mult,
                                    )
                                nc.vector.reduce_sum(
                                    ssum[:],
                                    am[:].rearrange("p (h j) -> p h j", h=2),
                                    AX.X,
                                )
                                rs = spool.tile([PT, 2], F32, tag="rs")
                                nc.vector.reciprocal(rs[:], ssum[:])
                                an = apool.tile([PT, 512], BF16, tag="an")
                                for hh in range(2):
                                    sl = slice(hh * JB, (hh + 1) * JB)
                                    nc.vector.tensor_scalar_mul(
                                        an[:, sl], am[:, sl], rs[:, hh:hh + 1])
                                atp = ps_at.tile([PT, 512], BF16, tag="atp")
                                for blk in range(4):
                                    bsl = slice(blk * PT, (blk + 1) * PT)
                                    nc.tensor.transpose(
                                        atp[:, bsl], an[:, bsl], iden_sb[:])
                                ats = apool.tile([PT, 512], BF16, tag="ats")
                                for blk in range(4):
                                    bsl = slice(blk * PT, (blk + 1) * PT)
                                    if blk % 2 == 0:
                                        nc.scalar.copy(ats[:, bsl], atp[:, bsl])
                                    else:
                                        nc.vector.tensor_copy(ats[:, bsl], atp[:, bsl])
                                cx = ps_cx.tile([PT, PT], F32, tag="cx")
                                for hh in range(2):
                                    for jb in range(2):
                                        nc.tensor.matmul(
                                            cx[hh * HD:(hh + 1) * HD, :],
                                            vs_sb[t + jb][:, (2 * p + hh) * HD:
                                                          (2 * p + hh + 1) * HD],
                                            ats[:, (2 * hh + jb) * PT:
                                                (2 * hh + jb + 1) * PT],
                                            start=(jb == 0), stop=(jb == 1),
                                            tile_position=(0, hh * HD),
                                        )
                                nc.scalar.copy(
                                    ctx_sb[p][:, t * PT:(t + 1) * PT], cx[:])

            # ---- Phase 4: out-proj, gating, blend, layernorm ----
            with (
                tc.tile_pool(name="stage8c", bufs=1) as spool8c,
                tc.tile_pool(name="oxpool", bufs=1) as oxpool,
                tc.tile_pool(name="ps4", bufs=4, space="PSUM") as ps4,
                tc.tile_pool(name="fin", bufs=2) as fin,
                tc.tile_pool(name="fstat", bufs=4) as fstat,
            ):
                wo_sb = [oxpool.tile([PT, H], BF16, tag=f"wo{i}", name=f"wo{i}")
                         for i in range(NT)]
                for i in range(NT):
                    unpack_w4(M_O, i, wo_sb[i], spool8c)
                xr_sb = [oxpool.tile([PT, H], BF16, tag=f"xr{i}", name=f"xr{i}")
                         for i in range(NT)]
                for i in range(NT):
                    nc.sync.dma_start(xr_sb[i][:], xn.ap()[i * PT:(i + 1) * PT, :])

                for st in range(NT):
                    y = fin.tile([PT, H], F32, tag="y")
                    for oh in range(2):
                        acc = ps4.tile([PT, 512], F32, tag="ps4")
                        for cp in range(NT):
                            nc.tensor.matmul(
                                acc[:],
                                ctx_sb[cp][:, st * PT:(st + 1) * PT],
                                wo_sb[cp][:, oh * 512:(oh + 1) * 512],
                                start=(cp == 0), stop=(cp == NT - 1),
                            )
                        sl = slice(oh * 512, (oh + 1) * 512)
                        if use_bo:
                            ob = fin.tile([PT, 512], F32, tag="ob")
                            nc.vector.tensor_tensor(
                                ob[:], acc[:], sm_sb[:, 1032 + oh * 512:1032 + (oh + 1) * 512],
                                op=ALU.add)
                            osrc = ob[:]
                        else:
                            osrc = acc[:]
                        m2 = fin.tile([PT, 512], F32, tag="m2")
                        nc.vector.tensor_tensor(
                            m2[:], t1_sb[st][:, sl], osrc, op=ALU.mult)
                        nc.vector.tensor_tensor(
                            y[:, sl], m2[:], xr_sb[st][:, sl], op=ALU.add)
                    # layernorm over the feature dim (free axis)
                    s1 = fstat.tile([PT, 1], F32, tag="s1")
                    nc.vector.reduce_sum(s1[:], y[:], axis=AX.X)
                    # square on DVE: keeps ACT pinned to the exp/tanh/ln
                    # table set (Square lives in another set -> ~1.3us
                    # ACT_TABLE_LOAD each time the sets alternate)
                    sq = fin.tile([PT, H], F32, tag="sq")
                    nc.vector.tensor_tensor(sq[:], y[:], y[:], op=ALU.mult)
                    s2 = fstat.tile([PT, 1], F32, tag="s2")
                    nc.vector.reduce_sum(s2[:], sq[:], axis=AX.X)
                    mu = fstat.tile([PT, 1], F32, tag="mu")
                    nc.vector.tensor_scalar_mul(mu[:], s1[:], 1.0 / H)
                    ey2 = fstat.tile([PT, 1], F32, tag="ey2")
                    nc.vector.tensor_scalar_mul(ey2[:], s2[:], 1.0 / H)
                    msq = fstat.tile([PT, 1], F32, tag="msq")
                    nc.vector.tensor_tensor(msq[:], mu[:], mu[:], op=ALU.mult)
                    var = fstat.tile([PT, 1], F32, tag="var")
                    nc.vector.tensor_tensor(var[:], ey2[:], msq[:], op=ALU.subtract)
                    # rstd = exp(-0.5 * ln(var + eps))   (stays in the exp/ln
                    # table set; Rsqrt activation is blocked for accuracy)
                    # y = 2*blended, so var_y = 4*var_blended: shift eps by 4x
                    vpe = fstat.tile([PT, 1], F32, tag="vpe")
                    nc.vector.tensor_scalar_add(vpe[:], var[:], 4.0 * LN_EPS)
                    lnv = fstat.tile([PT, 1], F32, tag="lnv")
                    nc.scalar.activation(lnv[:], vpe[:], AF.Ln)
                    rstd = fstat.tile([PT, 1], F32, tag="rstd")
                    nc.scalar.activation(rstd[:], lnv[:], AF.Exp, scale=-0.5)
                    mr = fstat.tile([PT, 1], F32, tag="mr")
                    nc.vector.tensor_tensor(mr[:], mu[:], rstd[:], op=ALU.mult)
                    nmr = fstat.tile([PT, 1], F32, tag="nmr")
                    nc.vector.tensor_scalar_mul(nmr[:], mr[:], -1.0)
                    res = fin.tile([PT, H], BF16, tag="res")
                    nc.scalar.activation(
                        res[:], y[:], AF.Identity,
                        bias=nmr[:], scale=rstd[:],
                    )
                    nc.sync.dma_start(outp.ap()[st * PT:(st + 1) * PT, :], res[:])

    nc.compile()
    return nc


def _get_program(use_bq: bool, use_bg: bool, use_bo: bool):
    key = (use_bq, use_bg, use_bo)
    if key not in _PROGRAM_CACHE:
        _PROGRAM_CACHE[key] = _build_program(*key)
    return _PROGRAM_CACHE[key]


def _make_masks() -> np.ndarray:
    # band mask for a 128-query tile vs its 256-wide key band; key j of
    # band col jj is global j = 128*t - 64 + jj, query i global = 128*t + i.
    i = np.arange(PT)[:, None]
    jj = np.arange(JB)[None, :]
    rel = jj - HW_ - i
    mid = (np.abs(rel) <= HW_)
    left = mid & (jj >= HW_)           # t == 0: j >= 0
    right = mid & (jj < JB - HW_)      # t == NT-1: j < S
    m = np.concatenate([left, mid, right], axis=1)
    return m.astype(NPBF16)


def kernel(**inputs) -> np.ndarray:
    inp = {k: np.asarray(v, dtype=np.float32) for k, v in inputs.items()}
    hidden, cross = inp["hidden_states"], inp["cross_states"]
    Wq, bq = inp["Wq"], inp["bq"]
    Wk = inp["Wk"]  # bk is not needed: it cancels in softmax
    Wv, bv = inp["Wv"], inp["bv"]
    Wo, bo = inp["Wo"], inp["bo"]
    Wg, bg = inp["Wg"], inp["bg"]
    ln_g, ln_b = inp["ln_g"], inp["ln_b"]

    bo_eff = bo + Wo @ bv
    use_bq = bool(np.any(bq != 0.0))
    use_bg = bool(np.any(bg != 0.0))
    use_bo = bool(np.any(bo_eff != 0.0))
    nc = _get_program(use_bq, use_bg, use_bo)

    # int4 pack: per-in-feature-row scale s (stored e3m4 x64), nibble pairs
    # over out-column pairs: byte = (n[2f] << 4) | n[2f+1]
    NPU8 = np.uint8
    w4_blocks, s_rows = [], []
    for W in (Wq, Wk, Wv, Wg, Wo):
        WT = np.ascontiguousarray(W.T).astype(np.float32)
        s = np.abs(WT).max(axis=1, keepdims=True) / 7.5
        s = np.maximum(s, 1e-8)
        s_q = (s * WSCALE).astype(NPFP8)
        s_rows.append(s_q.reshape(1, H))
        s_dev = s_q.astype(np.float32) / WSCALE
        n = np.clip(np.round(WT / s_dev) + 8.0, 0.0, 15.0).astype(NPU8)
        w4_blocks.append((n[:, 0::2] << 4) | n[:, 1::2])
    w4 = np.concatenate(w4_blocks, axis=0)

    smalls = None
    if use_bq or use_bg or use_bo:
        smalls = np.zeros((PT, 2056), np.float32)
        smalls[:, 0:NT] = (SCALE * bq).reshape(NT, PT).T
        smalls[:, 8:8 + H] = np.tile(bg[None, :], (PT, 1))
        smalls[:, 1032:1032 + H] = np.tile(bo_eff[None, :], (PT, 1))

    in_maps = []
    for b in range(B):
        f8 = np.concatenate(
            [np.ascontiguousarray(cross[b].T).astype(NPFP8)] + s_rows, axis=0)
        m = {"xn": hidden[b].astype(NPBF16), "f8": f8, "w4": w4}
        if smalls is not None:
            m["smalls"] = smalls
        in_maps.append(m)

    global _last_in_maps
    _last_in_maps = in_maps
    res = run_bass_kernel_spmd(nc, in_maps, list(range(NCORES)))
    out = np.stack([res.results[i]["out"].astype(np.float32)
                    for i in range(NCORES)], axis=0)

    if np.any(ln_g != 1.0) or np.any(ln_b != 0.0):
        out = out * ln_g[None, None, :] + ln_b[None, None, :]
    return out.astype(np.float32)


# revision 8
# speedup vs baseline: 1.0201x; 1.0201x over previous
"""Trainium2 Bass kernel for a windowed cross-attention layer.

Math (per batch element b):
    q = hidden @ Wq.T + bq ; k = cross @ Wk.T + bk ; v = cross @ Wv.T + bv
    scores = (q @ k.T) * HD**-0.5  with |i-j| <= WINDOW//2 band mask
    attn = softmax(scores) ; ctx = attn @ v ; out = ctx @ Wo.T + bo
    gate = sigmoid(hidden @ Wg.T + bg)
    y = layernorm(0.5*hidden + 0.5*gate*out) * ln_g + ln_b
  (bk cancels in softmax; bv folds into bo_eff = bo + Wo @ bv; layernorm
   scale-invariance lets the kernel feed 2*blended with eps scaled 4x;
   sigmoid(z) = 0.5*tanh(z/2) + 0.5 keeps ACT in one table set.)

Host<->device transfer dominates per-execution cost in this environment
(~0.73 ms per MB per core of incompressible data; compute overlaps under
it), so inputs are packed/quantized into 3 tensors per core:

  - xn  bf16 [1024, 1024]: hidden, natural layout.  hidden^T for the Q/G
    projections is derived on-chip via DMA-transpose (XBAR); band masks
    and the transpose identity are generated on-chip (affine_select).
  - w4  uint8 [6144, 512]: int4 nibble pairs for Wq/Wk/Wv/Wg/Wo (W.T,
    quantized per in-feature row, out-column pairs per byte) and cross^T
    (quantized per feature row).  Dequantized on-chip to bf16 with
    per-partition scale mult-add (2 DVE bitvec + 2 Pool ops per tile).
  - f8  fp8e3m4 [6, 1024]: the dequant scale rows (x64 for weights, x8
    for cross; e3m4 max is 15.5).
  - output is written bf16 and upcast host-side.

End-to-end quantization error (verified against reference, seed 0):
rel err 8.8e-3 vs the 2e-2 gate.

Sharding: data-parallel over batch. B == 8 == n_cores, one batch element
per NeuronCore, weights replicated, no collectives.
"""

import numpy as np

import concourse.bacc as bacc
import concourse.mybir as mybir
from concourse import tile
from concourse.bass_utils import run_bass_kernel_spmd

B, S, H, NH = 8, 1024, 1024, 16
HD = H // NH            # 64
WIN = 128
HW_ = WIN // 2          # 64  (window half-width)
SCALE = float(HD) ** -0.5
NCORES = 8
PT = 128                # partition tile
NT = H // PT            # 8
KPAD = S + 2 * HW_      # 1152 (left/right zero pads for the key band)
JB = 2 * WIN            # 256: key-band width per 128-query tile
LN_EPS = 1e-5
WSCALE = 64.0           # weight scale rows stored x64 in e3m4
WINV = float(1.0 / WSCALE)
NMAT = 5                # q, k, v, g, o (weights); ct is int4 block 5
NBLK = 6
CTSCALE = 8.0           # ct scale rows stored x8 (e3m4 max is 15.5)

F32 = mybir.dt.float32
BF16 = mybir.dt.bfloat16
FP8 = mybir.dt.float8e3
U8 = mybir.dt.uint8
NPBF16 = mybir.dt.np(BF16)
NPFP8 = mybir.dt.np(FP8)

AF = mybir.ActivationFunctionType
ALU = mybir.AluOpType
AX = mybir.AxisListType

_PROGRAM_CACHE: dict = {}

# w4 pack row bases (int4, nibble-packed along free-dim pairs)
M_Q, M_K, M_V, M_G, M_O, M_CT = range(NBLK)


def _build_program(use_bq: bool, use_bg: bool, use_bo: bool):
    nc = bacc.Bacc("TRN2", target_bir_lowering=False, debug=False)

    xn = nc.dram_tensor("xn", [S, H], BF16, kind="ExternalInput")
    f8 = nc.dram_tensor("f8", [NBLK, H], FP8, kind="ExternalInput")
    w4 = nc.dram_tensor("w4", [NBLK * H, H // 2], U8, kind="ExternalInput")
    use_smalls = use_bq or use_bg or use_bo
    if use_smalls:
        # [:, 0:8] SCALE*bq per out-tile, [:, 8:1032] bg bcast, [:, 1032:2056] bo_eff bcast
        smalls = nc.dram_tensor("smalls", [PT, 2056], F32, kind="ExternalInput")
    outp = nc.dram_tensor("out", [S, H], BF16, kind="ExternalOutput")

    with tile.TileContext(nc) as tc:
        with (
            tc.tile_pool(name="consts", bufs=1) as cpool,
            tc.tile_pool(name="ctxp", bufs=1) as ctxpool,
            tc.tile_pool(name="t1p", bufs=1) as t1pool,
        ):
            # masks + identity are generated on-chip (affine band predicates)
            mask_sb = cpool.tile([PT, 3 * JB], BF16, tag="mask")
            mid = mask_sb[:, JB:2 * JB]
            nc.gpsimd.memset(mid, 1.0)
            # mid: valid iff 0 <= jj - i <= 128
            nc.gpsimd.affine_select(mid, mid, pattern=[[1, JB]], base=0,
                                    channel_multiplier=-1,
                                    compare_op=ALU.is_ge, fill=0.0)
            nc.gpsimd.affine_select(mid, mid, pattern=[[-1, JB]], base=WIN,
                                    channel_multiplier=1,
                                    compare_op=ALU.is_ge, fill=0.0)
            # left tile: also jj >= 64 ; right tile: also jj <= 191
            nc.gpsimd.affine_select(mask_sb[:, 0:JB], mid, pattern=[[1, JB]],
                                    base=-HW_, channel_multiplier=0,
                                    compare_op=ALU.is_ge, fill=0.0)
            nc.gpsimd.affine_select(mask_sb[:, 2 * JB:3 * JB], mid,
                                    pattern=[[-1, JB]], base=(JB - HW_ - 1),
                                    channel_multiplier=0,
                                    compare_op=ALU.is_ge, fill=0.0)
            iden_sb = cpool.tile([PT, PT], BF16, tag="iden")
            nc.gpsimd.memset(iden_sb[:], 1.0)
            nc.gpsimd.affine_select(iden_sb[:], iden_sb[:], pattern=[[1, PT]],
                                    base=0, channel_multiplier=-1,
                                    compare_op=ALU.is_ge, fill=0.0)
            nc.gpsimd.affine_select(iden_sb[:], iden_sb[:], pattern=[[-1, PT]],
                                    base=0, channel_multiplier=1,
                                    compare_op=ALU.is_ge, fill=0.0)
            # int4 dequant scales: s_sb[:, m*8+i] = scale for (block m, tile i)
            s8_sb = cpool.tile([PT, NBLK * NT], FP8, tag="s8")
            nc.sync.dma_start(
                s8_sb[:].rearrange("p (m i) -> p m i", m=NBLK),
                f8.ap()[0:NBLK, :].rearrange("m (i p) -> p m i", p=PT))
            s_sb = cpool.tile([PT, NBLK * NT], F32, tag="ssc")
            nc.gpsimd.tensor_scalar_mul(s_sb[:, 0:NMAT * NT], s8_sb[:, 0:NMAT * NT], WINV)
            nc.gpsimd.tensor_scalar_mul(s_sb[:, NMAT * NT:], s8_sb[:, NMAT * NT:], 1.0 / CTSCALE)
            m8_sb = cpool.tile([PT, NBLK * NT], F32, tag="m8sc")
            nc.gpsimd.tensor_scalar_mul(m8_sb[:], s_sb[:], -8.0)

            def unpack_w4(m, i, dst_tile, stpool):
                """w4[m] tile i (uint8 nibble pairs) -> dst bf16 [128, H].

                byte b = (n_hi << 4) | n_lo packs out-columns (2f, 2f+1);
                w = (n - 8) * s  with s per in-feature row (partition).
                """
                sA = s_sb[:, m * NT + i:m * NT + i + 1]
                mA = m8_sb[:, m * NT + i:m * NT + i + 1]
                u8t = stpool.tile([PT, H // 2], U8, tag="u8")
                nc.scalar.dma_start(
                    u8t[:], w4.ap()[m * H + i * PT:m * H + (i + 1) * PT, :])
                d2 = dst_tile[:].rearrange("p (f t) -> p t f", t=2)
                nib = stpool.tile([PT, H // 2], U8, tag="nib")
                nc.gpsimd.tensor_scalar(
                    nib[:], u8t[:], 4, None, op0=ALU.logical_shift_right)
                nc.gpsimd.tensor_scalar(
                    d2[:, 0:1, :], nib[:].rearrange("p (o f) -> p o f", o=1),
                    sA, mA, op0=ALU.mult, op1=ALU.add)
                nib2 = stpool.tile([PT, H // 2], U8, tag="nib")
                nc.gpsimd.tensor_scalar(
                    nib2[:], u8t[:], 15, None, op0=ALU.bitwise_and)
                nc.gpsimd.tensor_scalar(
                    d2[:, 1:2, :], nib2[:].rearrange("p (o f) -> p o f", o=1),
                    sA, mA, op0=ALU.mult, op1=ALU.add)
            if use_smalls:
                sm_sb = cpool.tile([PT, 2056], F32, tag="smalls")
                nc.sync.dma_start(sm_sb[:], smalls.ap()[:])

            ctx_sb = [ctxpool.tile([PT, S], BF16, tag=f"ctx{i}", name=f"ctx{i}")
                      for i in range(NT)]
            t1_sb = [t1pool.tile([PT, H], BF16, tag=f"t1_{i}", name=f"t1_{i}")
                     for i in range(NT)]

            with tc.tile_pool(name="kvpool", bufs=1) as kvpool:
                # K^T padded key band [feature, 64 | tokens | 64]
                kt_sb = [kvpool.tile([PT, KPAD], BF16, tag=f"kt{i}", name=f"kt{i}")
                         for i in range(NT)]
                # V in shifted tiling: vs[u] rows = tokens [128u-64, 128u+64)
                vs_sb = [kvpool.tile([PT, H], BF16, tag=f"vs{i}", name=f"vs{i}")
                         for i in range(NT + 1)]
                for i in range(NT):
                    nc.gpsimd.memset(kt_sb[i][:, 0:HW_], 0.0)
                    nc.gpsimd.memset(kt_sb[i][:, KPAD - HW_:KPAD], 0.0)
                nc.gpsimd.memset(vs_sb[0][0:HW_, :], 0.0)
                nc.gpsimd.memset(vs_sb[NT][PT - HW_:PT, :], 0.0)

                # ---- Phase 1: K = cross @ Wk.T (transposed), V (shifted) ----
                with (
                    tc.tile_pool(name="stage8", bufs=1) as spool8,
                    tc.tile_pool(name="ctpool", bufs=1) as ctpool,
                    tc.tile_pool(name="w1", bufs=1) as wpool1,
                    tc.tile_pool(name="ps1", bufs=4, space="PSUM") as ps1,
                ):
                    ct_sb = [ctpool.tile([PT, S], BF16, tag=f"ct{i}", name=f"ct{i}")
                             for i in range(NT)]
                    wk_sb = [wpool1.tile([PT, H], BF16, tag=f"wk{i}", name=f"wk{i}")
                             for i in range(NT)]
                    wv_sb = [wpool1.tile([PT, H], BF16, tag=f"wv{i}", name=f"wv{i}")
                             for i in range(NT)]
                    for i in range(NT):
                        unpack_w4(M_CT, i, ct_sb[i], spool8)
                        unpack_w4(M_K, i, wk_sb[i], spool8)
                        unpack_w4(M_V, i, wv_sb[i], spool8)

                    # K^T[o, s] = sum_h Wk.T[h, o].T @ cross^T[h, s]
                    for ot in range(NT):
                        for sh in range(2):
                            acc = ps1.tile([PT, 512], F32, tag="ps1")
                            for ht in range(NT):
                                nc.tensor.matmul(
                                    acc[:],
                                    wk_sb[ht][:, ot * PT:(ot + 1) * PT],
                                    ct_sb[ht][:, sh * 512:(sh + 1) * 512],
                                    start=(ht == 0), stop=(ht == NT - 1),
                                )
                            nc.scalar.copy(
                                kt_sb[ot][:, HW_ + sh * 512: HW_ + (sh + 1) * 512],
                                acc[:],
                            )

                    # V[s, o] = cross @ Wv.T, then build the token-shifted
                    # tiles via SBUF->SBUF DMA (compute engines cannot move
                    # data across partition lanes).
                    v_sb = [ctpool.tile([PT, H], BF16, tag=f"v{i}", name=f"v{i}")
                            for i in range(NT)]
                    for st in range(NT):
                        for oh in range(2):
                            acc = ps1.tile([PT, 512], F32, tag="ps1")
                            for ht in range(NT):
                                nc.tensor.matmul(
                                    acc[:],
                                    ct_sb[ht][:, st * PT:(st + 1) * PT],
                                    wv_sb[ht][:, oh * 512:(oh + 1) * 512],
                                    start=(ht == 0), stop=(ht == NT - 1),
                                )
                            nc.scalar.copy(
                                v_sb[st][:, oh * 512:(oh + 1) * 512], acc[:])
                    for u in range(NT + 1):
                        if u > 0:
                            nc.sync.dma_start(
                                vs_sb[u][0:HW_, :], v_sb[u - 1][HW_:PT, :])
                        if u < NT:
                            nc.sync.dma_start(
                                vs_sb[u][HW_:PT, :], v_sb[u][0:HW_, :])

                with tc.tile_pool(name="qpool", bufs=1) as qpool:
                    qt_sb = [qpool.tile([PT, S], BF16, tag=f"qt{i}", name=f"qt{i}")
                             for i in range(NT)]

                    # ---- Phase 2: Q^T (scaled, biased) and gate tanh ----
                    with (
                        tc.tile_pool(name="stage8b", bufs=1) as spool8b,
                        tc.tile_pool(name="xtpool", bufs=1) as xtpool,
                        tc.tile_pool(name="w2", bufs=1) as wpool2,
                        tc.tile_pool(name="ps2", bufs=4, space="PSUM") as ps2,
                        tc.tile_pool(name="gtmp", bufs=3) as gtmp,
                    ):
                        # hidden^T via DMA-transpose from the natural-layout rows
                        xt_sb = [xtpool.tile([PT, S], BF16, tag=f"xt{i}", name=f"xt{i}")
                                 for i in range(NT)]
                        for i in range(NT):
                            nc.sync.dma_start(
                                xt_sb[i][:], xn.ap()[0:S, i * PT:(i + 1) * PT],
                                transpose=True)
                        wq_sb = [wpool2.tile([PT, H], BF16, tag=f"wq{i}", name=f"wq{i}")
                                 for i in range(NT)]
                        wg_sb = [wpool2.tile([PT, H], BF16, tag=f"wg{i}", name=f"wg{i}")
                                 for i in range(NT)]
                        for i in range(NT):
                            unpack_w4(M_Q, i, wq_sb[i], spool8b)
                            unpack_w4(M_G, i, wg_sb[i], spool8b)

                        for ot in range(NT):
                            for sh in range(2):
                                acc = ps2.tile([PT, 512], F32, tag="ps2")
                                for ht in range(NT):
                                    nc.tensor.matmul(
                                        acc[:],
                                        wq_sb[ht][:, ot * PT:(ot + 1) * PT],
                                        xt_sb[ht][:, sh * 512:(sh + 1) * 512],
                                        start=(ht == 0), stop=(ht == NT - 1),
                                    )
                                # q_scaled = SCALE*q (+ SCALE*bq)
                                nc.scalar.activation(
                                    qt_sb[ot][:, sh * 512:(sh + 1) * 512],
                                    acc[:], AF.Identity,
                                    bias=(sm_sb[:, ot:ot + 1] if use_bq else 0.0),
                                    scale=SCALE,
                                )

                        # z[s, o] = hidden @ Wg.T ; t1 = sigmoid(z) via tanh
                        for st in range(NT):
                            for oh in range(2):
                                acc = ps2.tile([PT, 512], F32, tag="ps2")
                                for ht in range(NT):
                                    nc.tensor.matmul(
                                        acc[:],
                                        xt_sb[ht][:, st * PT:(st + 1) * PT],
                                        wg_sb[ht][:, oh * 512:(oh + 1) * 512],
                                        start=(ht == 0), stop=(ht == NT - 1),
                                    )
                                sl = slice(oh * 512, (oh + 1) * 512)
                                if use_bg:
                                    zb = gtmp.tile([PT, 512], F32, tag="zb")
                                    nc.vector.tensor_tensor(
                                        zb[:], acc[:], sm_sb[:, 8 + oh * 512:8 + (oh + 1) * 512],
                                        op=ALU.add)
                                    zin = zb
                                else:
                                    zin = acc
                                th = gtmp.tile([PT, 512], BF16, tag="th")
                                nc.scalar.activation(th[:], zin[:], AF.Tanh, scale=0.5)
                                # gate = sigmoid(z) = 0.5*tanh(z/2) + 0.5
                                nc.vector.tensor_scalar(
                                    t1_sb[st][:, sl], th[:], 0.5, 0.5,
                                    op0=ALU.mult, op1=ALU.add)

                    # ---- Phase 3: windowed attention ----
                    with (
                        tc.tile_pool(name="attn_sb", bufs=3) as apool,
                        tc.tile_pool(name="stats", bufs=4) as spool,
                        tc.tile_pool(name="ps_sc", bufs=2, space="PSUM") as ps_sc,
                        tc.tile_pool(name="ps_at", bufs=2, space="PSUM") as ps_at,
                        tc.tile_pool(name="ps_cx", bufs=2, space="PSUM") as ps_cx,
                    ):
                        for p in range(NT):
                            for t in range(NT):   # query tile
                                mv = 0 if t == 0 else (2 if t == NT - 1 else 1)
                                # separate PSUM tiles per head: the two MMs
                                # use disjoint PE row-groups (partition base
                                # 0 vs 64) and can run concurrently in the
                                # array — concurrent writes to one PSUM bank
                                # are fatal on HW.
                                scs = [ps_sc.tile([PT, JB], F32, tag=f"sc{h}",
                                                  name=f"sc{h}")
                                       for h in range(2)]
                                for hh in range(2):
                                    nc.tensor.matmul(
                                        scs[hh][:],
                                        qt_sb[p][hh * HD:(hh + 1) * HD,
                                                 t * PT:(t + 1) * PT],
                                        kt_sb[p][hh * HD:(hh + 1) * HD,
                                                 t * PT:t * PT + JB],
                                        start=True, stop=True,
                                    )
                                ex = apool.tile([PT, 512], BF16, tag="ex")
                                for hh in range(2):
                                    nc.scalar.activation(
                                        ex[:, hh * JB:(hh + 1) * JB],
                                        scs[hh][:], AF.Exp)
                                am = apool.tile([PT, 512], BF16, tag="am")
                                ssum = spool.tile([PT, 2], F32, tag="ssum")
                                for hh in range(2):
                                    sl = slice(hh * JB, (hh + 1) * JB)
                                    nc.vector.tensor_tensor(
                                        am[:, sl], ex[:, sl],
                                        mask_sb[:, mv * JB:(mv + 1) * JB],
                                        op=ALU.mult,
                                    )
                                nc.vector.reduce_sum(
                                    ssum[:],
                                    am[:].rearrange("p (h j) -> p h j", h=2),
                                    AX.X,
                                )
                                rs = spool.tile([PT, 2], F32, tag="rs")
                                nc.vector.reciprocal(rs[:], ssum[:])
                                an = apool.tile([PT, 512], BF16, tag="an")
                                for hh in range(2):
                                    sl = slice(hh * JB, (hh + 1) * JB)
                                    nc.vector.tensor_scalar_mul(
                                        an[:, sl], am[:, sl], rs[:, hh:hh + 1])
                                atp = ps_at.tile([PT, 512], BF16, tag="atp")
                                for blk in range(4):
                                    bsl = slice(blk * PT, (blk + 1) * PT)
                                    nc.tensor.transpose(
                                        atp[:, bsl], an[:, bsl], iden_sb[:])
                                ats = apool.tile([PT, 512], BF16, tag="ats")
                                for blk in range(4):
                                    bsl = slice(blk * PT, (blk + 1) * PT)
                                    if blk % 2 == 0:
                                        nc.scalar.copy(ats[:, bsl], atp[:, bsl])
                                    else:
                                        nc.vector.tensor_copy(ats[:, bsl], atp[:, bsl])
                                cx = ps_cx.tile([PT, PT], F32, tag="cx")
                                for hh in range(2):
                                    for jb in range(2):
                                        nc.tensor.matmul(
                                            cx[hh * HD:(hh + 1) * HD, :],
                                            vs_sb[t + jb][:, (2 * p + hh) * HD:
                                                          (2 * p + hh + 1) * HD],
                                            ats[:, (2 * hh + jb) * PT:
                                                (2 * hh + jb + 1) * PT],
                                            start=(jb == 0), stop=(jb == 1),
                                            tile_position=(0, hh * HD),
                                        )
                                nc.scalar.copy(
                                    ctx_sb[p][:, t * PT:(t + 1) * PT], cx[:])

            # ---- Phase 4: out-proj, gating, blend, layernorm ----
            with (
                tc.tile_pool(name="stage8c", bufs=1) as spool8c,
                tc.tile_pool(name="oxpool", bufs=1) as oxpool,
                tc.tile_pool(name="ps4", bufs=4, space="PSUM") as ps4,
                tc.tile_pool(name="fin", bufs=2) as fin,
                tc.tile_pool(name="fstat", bufs=4) as fstat,
            ):
                wo_sb = [oxpool.tile([PT, H], BF16, tag=f"wo{i}", name=f"wo{i}")
                         for i in range(NT)]
                for i in range(NT):
                    unpack_w4(M_O, i, wo_sb[i], spool8c)
                xr_sb = [oxpool.tile([PT, H], BF16, tag=f"xr{i}", name=f"xr{i}")
                         for i in range(NT)]
                for i in range(NT):
                    nc.sync.dma_start(xr_sb[i][:], xn.ap()[i * PT:(i + 1) * PT, :])

                for st in range(NT):
                    y = fin.tile([PT, H], F32, tag="y")
                    for oh in range(2):
                        acc = ps4.tile([PT, 512], F32, tag="ps4")
                        for cp in range(NT):
                            nc.tensor.matmul(
                                acc[:],
                                ctx_sb[cp][:, st * PT:(st + 1) * PT],
                                wo_sb[cp][:, oh * 512:(oh + 1) * 512],
                                start=(cp == 0), stop=(cp == NT - 1),
                            )
                        sl = slice(oh * 512, (oh + 1) * 512)
                        if use_bo:
                            ob = fin.tile([PT, 512], F32, tag="ob")
                            nc.vector.tensor_tensor(
                                ob[:], acc[:], sm_sb[:, 1032 + oh * 512:1032 + (oh + 1) * 512],
                                op=ALU.add)
                            osrc = ob[:]
                        else:
                            osrc = acc[:]
                        m2 = fin.tile([PT, 512], F32, tag="m2")
                        nc.vector.tensor_tensor(
                            m2[:], t1_sb[st][:, sl], osrc, op=ALU.mult)
                        nc.vector.tensor_tensor(
                            y[:, sl], m2[:], xr_sb[st][:, sl], op=ALU.add)
                    # layernorm over the feature dim (free axis)
                    s1 = fstat.tile([PT, 1], F32, tag="s1")
                    nc.vector.reduce_sum(s1[:], y[:], axis=AX.X)
                    # square on DVE: keeps ACT pinned to the exp/tanh/ln
                    # table set (Square lives in another set -> ~1.3us
                    # ACT_TABLE_LOAD each time the sets alternate)
                    sq = fin.tile([PT, H], F32, tag="sq")
                    nc.vector.tensor_tensor(sq[:], y[:], y[:], op=ALU.mult)
                    s2 = fstat.tile([PT, 1], F32, tag="s2")
                    nc.vector.reduce_sum(s2[:], sq[:], axis=AX.X)
                    mu = fstat.tile([PT, 1], F32, tag="mu")
                    nc.vector.tensor_scalar_mul(mu[:], s1[:], 1.0 / H)
                    ey2 = fstat.tile([PT, 1], F32, tag="ey2")
                    nc.vector.tensor_scalar_mul(ey2[:], s2[:], 1.0 / H)
                    msq = fstat.tile([PT, 1], F32, tag="msq")
                    nc.vector.tensor_tensor(msq[:], mu[:], mu[:], op=ALU.mult)
                    var = fstat.tile([PT, 1], F32, tag="var")
                    nc.vector.tensor_tensor(var[:], ey2[:], msq[:], op=ALU.subtract)
                    # rstd = exp(-0.5 * ln(var + eps))   (stays in the exp/ln
                    # table set; Rsqrt activation is blocked for accuracy)
                    # y = 2*blended, so var_y = 4*var_blended: shift eps by 4x
                    vpe = fstat.tile([PT, 1], F32, tag="vpe")
                    nc.vector.tensor_scalar_add(vpe[:], var[:], 4.0 * LN_EPS)
                    lnv = fstat.tile([PT, 1], F32, tag="lnv")
                    nc.scalar.activation(lnv[:], vpe[:], AF.Ln)
                    rstd = fstat.tile([PT, 1], F32, tag="rstd")
                    nc.scalar.activation(rstd[:], lnv[:], AF.Exp, scale=-0.5)
                    mr = fstat.tile([PT, 1], F32, tag="mr")
                    nc.vector.tensor_tensor(mr[:], mu[:], rstd[:], op=ALU.mult)
                    nmr = fstat.tile([PT, 1], F32, tag="nmr")
                    nc.vector.tensor_scalar_mul(nmr[:], mr[:], -1.0)
                    res = fin.tile([PT, H], BF16, tag="res")
                    nc.scalar.activation(
                        res[:], y[:], AF.Identity,
                        bias=nmr[:], scale=rstd[:],
                    )
                    nc.sync.dma_start(outp.ap()[st * PT:(st + 1) * PT, :], res[:])

    nc.compile()
    return nc


def _get_program(use_bq: bool, use_bg: bool, use_bo: bool):
    key = (use_bq, use_bg, use_bo)
    if key not in _PROGRAM_CACHE:
        _PROGRAM_CACHE[key] = _build_program(*key)
    return _PROGRAM_CACHE[key]


def _make_masks() -> np.ndarray:
    # band mask for a 128-query tile vs its 256-wide key band; key j of
    # band col jj is global j = 128*t - 64 + jj, query i global = 128*t + i.
    i = np.arange(PT)[:, None]
    jj = np.arange(JB)[None, :]
    rel = jj - HW_ - i
    mid = (np.abs(rel) <= HW_)
    left = mid & (jj >= HW_)           # t == 0: j >= 0
    right = mid & (jj < JB - HW_)      # t == NT-1: j < S
    m = np.concatenate([left, mid, right], axis=1)
    return m.astype(NPBF16)


def kernel(**inputs) -> np.ndarray:
    inp = {k: np.asarray(v, dtype=np.float32) for k, v in inputs.items()}
    hidden, cross = inp["hidden_states"], inp["cross_states"]
    Wq, bq = inp["Wq"], inp["bq"]
    Wk = inp["Wk"]  # bk is not needed: it cancels in softmax
    Wv, bv = inp["Wv"], inp["bv"]
    Wo, bo = inp["Wo"], inp["bo"]
    Wg, bg = inp["Wg"], inp["bg"]
    ln_g, ln_b = inp["ln_g"], inp["ln_b"]

    bo_eff = bo + Wo @ bv
    use_bq = bool(np.any(bq != 0.0))
    use_bg = bool(np.any(bg != 0.0))
    use_bo = bool(np.any(bo_eff != 0.0))
    nc = _get_program(use_bq, use_bg, use_bo)

    # int4 pack: per-in-feature-row scale s (stored e3m4 x64), nibble pairs
    # over out-column pairs: byte = (n[2f] << 4) | n[2f+1]
    NPU8 = np.uint8

    def int4_pack(M, sstore):
        s = np.abs(M).max(axis=1, keepdims=True) / 7.5
        s = np.maximum(s, 1e-8)
        s_q = (s * sstore).astype(NPFP8)
        s_dev = s_q.astype(np.float32) / sstore
        n = np.clip(np.round(M / s_dev) + 8.0, 0.0, 15.0).astype(NPU8)
        return (n[:, 0::2] << 4) | n[:, 1::2], s_q.reshape(1, H)

    w4_blocks, s_rows = [], []
    for W in (Wq, Wk, Wv, Wg, Wo):
        blk, s_q = int4_pack(np.ascontiguousarray(W.T).astype(np.float32), WSCALE)
        w4_blocks.append(blk)
        s_rows.append(s_q)
    w4_w = np.concatenate(w4_blocks, axis=0)

    smalls = None
    if use_bq or use_bg or use_bo:
        smalls = np.zeros((PT, 2056), np.float32)
        smalls[:, 0:NT] = (SCALE * bq).reshape(NT, PT).T
        smalls[:, 8:8 + H] = np.tile(bg[None, :], (PT, 1))
        smalls[:, 1032:1032 + H] = np.tile(bo_eff[None, :], (PT, 1))

    in_maps = []
    for b in range(B):
        ct_blk, ct_s = int4_pack(
            np.ascontiguousarray(cross[b].T).astype(np.float32), CTSCALE)
        f8 = np.concatenate(s_rows + [ct_s], axis=0)
        m = {"xn": hidden[b].astype(NPBF16), "f8": f8,
             "w4": np.concatenate([w4_w, ct_blk], axis=0)}
        if smalls is not None:
            m["smalls"] = smalls
        in_maps.append(m)

    global _last_in_maps
    _last_in_maps = in_maps
    res = run_bass_kernel_spmd(nc, in_maps, list(range(NCORES)))
    out = np.stack([res.results[i]["out"].astype(np.float32)
                    for i in range(NCORES)], axis=0)

    if np.any(ln_g != 1.0) or np.any(ln_b != 0.0):
        out = out * ln_g[None, None, :] + ln_b[None, None, :]
    return out.astype(np.float32)
